# revision 17
# baseline (speedup 1.0000x reference)
"""DCGRU cell on 8 Trainium2 NeuronCores — fp8 DoubleRow edition.

Sharding: data-parallel over batch (B=32 -> 4 per core), adjacency + MLP
weights replicated. No collectives; host gathers per-core outputs.

Key ideas vs the bf16 baseline:
  * Diffusion hop matmuls run in fp8e4 with MatmulPerfMode.DoubleRow: each
    instruction contracts TWO 128-row k-tiles (lhsT [128,2,M], rhs [128,2,N])
    at 0.5 cycles/output-col — 2-4x the bf16 rate.
  * Diffusion 2 only propagates the r*h feature columns (128/batch instead of
    192): the x-part hop features are identical to diffusion 1's and are
    reused for the n-gate MLP. Saves 1/3 of diffusion-2 hop FLOPs.
  * Gate logits accumulate across all 7 k-blocks in a single PSUM group per
    (batch, 512-col block) — no DRAM accumulators, no accum DMAs. Hop
    features spill to DRAM (fp8) and are gathered back per block in paired
    DoubleRow layout.
  * The k=0 MLP segments (the raw x_h / rh features, which dominate logit
    magnitude) stay bf16 with weights pre-scaled by 8192 so they share the
    PSUM accumulation group with the fp8 hop segments.

Scaling scheme (fp8e4 max normal 240):
  x_h, rh stored *16; W stored *512; hop feats stored *128.
  hop1 psum = 16*512*hop  -> copy scale 1/64  -> *128
  hopk psum = 128*512*hop -> copy scale 1/512 -> *128
  MLP hop weights *64 -> logit psum = 128*64 = 8192*logit
  k0 weights: *8192 (vs raw x_h bf16), *512 (vs rh16 bf16)
  activation scale 1/8192 recovers logits.

Per-batch feature order matches the reference concat:
  k-blocks [x_h, Wf^1, Wf^2, Wf^3, Wb^1, Wb^2, Wb^3], 192 feats each.
"""

import sys
import numpy as np
import ml_dtypes

for _p in ("/opt/trn_rl_repo",):
    if _p not in sys.path:
        sys.path.insert(0, _p)

from concourse import bacc, tile, mybir  # noqa: E402
from concourse.bass_utils import run_bass_kernel_spmd  # noqa: E402

F32 = mybir.dt.float32
BF16 = mybir.dt.bfloat16
FP8 = mybir.dt.float8e4
AF = mybir.ActivationFunctionType
DR = mybir.MatmulPerfMode.DoubleRow
E4 = ml_dtypes.float8_e4m3
BF = ml_dtypes.bfloat16

C = 4            # batches per core
FI = 192         # per-batch feature width in d1 (x 64 + h 128)
DH = 128
NCORES = 8
NHOPS = 3
NJP = 8          # node-tile pairs (N = NJP*256)
NBK = 4          # 512-col node blocks
SX = 16.0        # x_h / rh fp8 scale
SW = 512.0       # W fp8 scale
SF = 128.0       # hop-feature fp8 scale
SMW = 64.0       # MLP hop-weight fp8 scale
SPS = SF * SMW   # logit psum scale (8192)


def build_nc():
    N = NJP * 256
    nc = bacc.Bacc("TRN2", target_bir_lowering=False, debug=False,
                   num_devices=NCORES)

    def din(name, shape, dt=F32):
        return nc.dram_tensor(name, shape, dt, kind="ExternalInput").ap()

    XHNM = din("xh_nm8", [NJP, 128, 2, 768], FP8)     # 16*x_h node-major paired
    XHK0 = din("xh_k0", [C, FI, N], BF16)             # x_h feature-major raw
    WFP = din("wfp", [NJP, 128, 2, N], FP8)           # 512*W_fwd^T paired
    WBP = din("wbp", [NJP, 128, 2, N], FP8)
    HFM = din("h_fm", [C, DH, N])                     # h_prev feature-major f32
    # MLP weights (see packer)
    WK0 = {g: din(f"w{g}k0", [FI, DH], BF16) for g in ("r", "z")}
    WA = {g: din(f"w{g}A", [3, 128, 2, DH], FP8) for g in ("r", "z")}
    WB = {g: din(f"w{g}B", [3, 64, 2, DH], FP8) for g in ("r", "z")}
    WNK0X = din("wnk0x", [64, DH], BF16)
    WNK0R = din("wnk0r", [DH, DH], BF16)
    WNX = din("wnx", [3, 64, 2, DH], FP8)
    WNR = din("wnr", [3, 128, 2, DH], FP8)
    BRT = din("br_c", [DH, 1])
    BZT = din("bz_c", [DH, 1])
    BNT = din("bn_c", [DH, 1])
    IDB = din("identb", [128, 128], BF16)
    OUT = nc.dram_tensor("out_fm", [C, DH, N], F32, kind="ExternalOutput").ap()

    # k-pair-adjacent spill layouts: [pair][rows][h][N] so MLP gathers are
    # single 3-D DMAs
    FEAT1 = nc.dram_tensor("feat1", [3, 768, 2, N], FP8).ap()   # d1 feats *128
    FEAT2 = nc.dram_tensor("feat2", [3, C * DH, 2, N], FP8).ap()  # d2 feats

    with tile.TileContext(nc) as tc:
        with (
            tc.tile_pool(name="const", bufs=1) as cpool,
            tc.tile_pool(name="nmx", bufs=8) as nmx_pool,
            tc.tile_pool(name="nm1", bufs=10) as nm1_pool,
            tc.tile_pool(name="nm2", bufs=10) as nm2_pool,
            tc.tile_pool(name="wc", bufs=8) as wc_pool,
            tc.tile_pool(name="fm1", bufs=12) as fm1_pool,
            tc.tile_pool(name="fm2", bufs=8) as fm2_pool,
            tc.tile_pool(name="stg", bufs=9) as stg_pool,
            tc.tile_pool(name="feed", bufs=16) as feed_pool,
            tc.tile_pool(name="k0p", bufs=4) as k0_pool,
            tc.tile_pool(name="gw", bufs=2) as gw_pool,
            tc.tile_pool(name="gres", bufs=4) as gres_pool,
            tc.tile_pool(name="ps", bufs=6, space="PSUM") as ps_pool,
            tc.tile_pool(name="pst", bufs=2, space="PSUM") as pst_pool,
        ):
            # ---------------- constants ----------------
            identb = cpool.tile([128, 128], BF16, tag="idb")
            nc.scalar.dma_start(identb[:], IDB[:])
            # resident paired node-major x_h (chain start for both dirs)
            nm_xh = []
            _qs = [nc.sync, nc.scalar, nc.gpsimd]
            for jp in range(NJP):
                t = nmx_pool.tile([128, 2, 768], FP8, name="t", tag="nmx")
                _qs[jp % 3].dma_start(t[:], XHNM[jp])
                nm_xh.append(t)

            def load_wdir(WP):
                """Block-major striped load: all jp's block-0 slices land
                first so the next hop's first 512-col block starts after
                ~1MB instead of the full 4.2MB."""
                ws = []
                qs = [nc.sync, nc.scalar, nc.gpsimd]
                for jp in range(NJP):
                    t = wc_pool.tile([128, 2, N], FP8, name="t", tag="w")
                    ws.append(t)
                for blk in range(NBK):
                    cs = slice(512 * blk, 512 * (blk + 1))
                    for jp in range(NJP):
                        qs[jp % 3].dma_start(ws[jp][:, :, cs],
                                             WP[jp][:, :, cs])
                return ws

            def hop(cur, ws, nch, k, FEATD, kh, rowbase):
                """One DoubleRow hop with fused per-block retransposition.
                Spills fp8 *SF feats to FEATD[:, kh, :]; for k < NHOPS also
                returns the next chain's paired fp8 nm tiles (transposes for
                jp pair (2b, 2b+1) only need block b's columns)."""
                nms = [] if k < NHOPS else None
                scale = 1.0 / 64.0 if k == 1 else 1.0 / 512.0
                for blk in range(NBK):
                    cs = slice(512 * blk, 512 * (blk + 1))
                    pss = [ps_pool.tile([128, 512], F32, name="t", tag="hop")
                           for _ in range(nch)]
                    for jp in range(NJP):
                        rhs = ws[jp][:, :, cs]
                        for c in range(nch):
                            nc.tensor.matmul(
                                pss[c][:],
                                cur[jp][:, :, 128 * c:128 * (c + 1)],
                                rhs,
                                start=(jp == 0), stop=(jp == NJP - 1),
                                perf_mode=DR)
                    fmb = None
                    if nms is not None:
                        pool = fm1_pool if nch == 6 else fm2_pool
                        tg = "fm1" if nch == 6 else "fm2"
                        fmb = [pool.tile([128, 512], BF16, name="t", tag=tg)
                               for _ in range(nch)]
                    for c in range(nch):
                        stg = stg_pool.tile([128, 512], FP8, name="t",
                                            tag="stg")
                        nc.scalar.activation(stg[:], pss[c][:], AF.Copy,
                                             scale=scale)
                        nc.sync.dma_start(
                            FEATD[rowbase + 128 * c:rowbase + 128 * (c + 1),
                                  kh, cs], stg[:])
                        if fmb is not None:
                            nc.vector.tensor_scalar_mul(fmb[c][:], pss[c][:],
                                                        scale)
                    if nms is not None:
                        for q in range(2):
                            t = (nm1_pool.tile([128, 2, 768], FP8, name="t",
                                               tag="nm1") if nch == 6 else
                                 nm2_pool.tile([128, 2, 512], FP8, name="t",
                                               tag="nm2"))
                            for h in range(2):
                                ps = pst_pool.tile([128, 128 * nch], BF16,
                                                   name="t", tag="tr")
                                for c in range(nch):
                                    nc.tensor.transpose(
                                        ps[:, 128 * c:128 * (c + 1)],
                                        fmb[c][:, 128 * (2 * q + h):
                                               128 * (2 * q + h + 1)],
                                        identb[:])
                                nc.vector.tensor_copy(t[:, h, :], ps[:])
                            nms.append(t)
                return nms

            # ---------------- diffusion 1 ----------------
            with nc.named_scope("d1_hops"):
                for dirw, WP in ((0, WFP), (1, WBP)):
                    ws = load_wdir(WP)
                    cur = nm_xh
                    for k in range(1, NHOPS + 1):
                        kidx = dirw * NHOPS + k  # 1..6
                        nxt = hop(cur, ws, 6, k,
                                  FEAT1[(kidx - 1) // 2], (kidx - 1) % 2, 0)
                        if k < NHOPS:
                            cur = nxt

            # ---- deferred small-const loads (weights/biases for MLPs) ----
            brt = cpool.tile([DH, 1], F32, tag="brt")
            nc.scalar.dma_start(brt[:], BRT[:])
            bzt = cpool.tile([DH, 1], F32, tag="bzt")
            nc.scalar.dma_start(bzt[:], BZT[:])
            bnt = cpool.tile([DH, 1], F32, tag="bnt")
            nc.scalar.dma_start(bnt[:], BNT[:])
            wk0a, wk0b, wa, wb = {}, {}, {}, {}
            for g in ("r", "z"):
                wk0a[g] = cpool.tile([128, DH], BF16, name="t", tag=f"w{g}k0a")
                nc.scalar.dma_start(wk0a[g][:], WK0[g][0:128, :])
                wk0b[g] = cpool.tile([64, DH], BF16, name="t", tag=f"w{g}k0b")
                nc.scalar.dma_start(wk0b[g][:], WK0[g][128:192, :])
                wa[g] = []
                wb[g] = []
                for p in range(3):
                    t = cpool.tile([128, 2, DH], FP8, name="t", tag=f"w{g}A{p}")
                    nc.scalar.dma_start(t[:], WA[g][p])
                    wa[g].append(t)
                    t = cpool.tile([64, 2, DH], FP8, name="t", tag=f"w{g}B{p}")
                    nc.scalar.dma_start(t[:], WB[g][p])
                    wb[g].append(t)
            wnk0x = cpool.tile([64, DH], BF16, tag="wnk0x")
            nc.scalar.dma_start(wnk0x[:], WNK0X[:])
            wnk0r = cpool.tile([DH, DH], BF16, tag="wnk0r")
            nc.scalar.dma_start(wnk0r[:], WNK0R[:])
            wnx, wnr = [], []
            for p in range(3):
                t = cpool.tile([64, 2, DH], FP8, name="t", tag=f"wnx{p}")
                nc.scalar.dma_start(t[:], WNX[p])
                wnx.append(t)
                t = cpool.tile([128, 2, DH], FP8, name="t", tag=f"wnr{p}")
                nc.scalar.dma_start(t[:], WNR[p])
                wnr.append(t)


            # ---------------- MLP r,z + rh ----------------
            sc_rz = nc.enter_named_scope("mlp_rz", False)
            z16 = [gres_pool.tile([DH, N], BF16, name="t", tag="z16")
                   for _ in range(C)]
            rh16 = [gres_pool.tile([DH, N], BF16, name="t", tag="rh16")
                    for _ in range(C)]
            for b in range(C):
                for blk in range(NBK):
                    cs = slice(512 * blk, 512 * (blk + 1))
                    k0a = k0_pool.tile([128, 512], BF16, name="t", tag="k0a")
                    nc.gpsimd.dma_start(k0a[:], XHK0[b][0:128, cs])
                    k0b = k0_pool.tile([64, 512], BF16, name="t", tag="k0b")
                    nc.gpsimd.dma_start(k0b[:], XHK0[b][128:192, cs])
                    fA, fB = [], []
                    for p in range(3):
                        tA = feed_pool.tile([128, 2, 512], FP8, name="t",
                                            tag="fA")
                        nc.gpsimd.dma_start(
                            tA[:], FEAT1[p][b * FI:b * FI + 128, :, cs])
                        fA.append(tA)
                        tB = feed_pool.tile([64, 2, 512], FP8, name="t",
                                            tag="fB")
                        nc.scalar.dma_start(
                            tB[:], FEAT1[p][b * FI + 128:b * FI + 192, :, cs])
                        fB.append(tB)
                    hblk = gw_pool.tile([DH, 512], F32, name="t", tag="h")
                    nc.sync.dma_start(hblk[:], HFM[b][:, cs])
                    for g in ("r", "z"):
                        ps = ps_pool.tile([128, 512], F32, name="t", tag="hop")
                        nc.tensor.matmul(ps[:], wk0a[g][:], k0a[:],
                                         start=True, stop=False)
                        nc.tensor.matmul(ps[:], wk0b[g][:], k0b[:],
                                         start=False, stop=False)
                        for p in range(3):
                            nc.tensor.matmul(ps[:], wa[g][p][:], fA[p][:],
                                             start=False, stop=False,
                                             perf_mode=DR)
                            nc.tensor.matmul(ps[:], wb[g][p][:], fB[p][:],
                                             start=False, stop=(p == 2),
                                             perf_mode=DR)
                        if g == "r":
                            rwk = gw_pool.tile([DH, 512], F32, name="t",
                                               tag="rw")
                            nc.scalar.activation(rwk[:], ps[:], AF.Sigmoid,
                                                 bias=brt[:], scale=1.0 / SPS)
                            nc.vector.scalar_tensor_tensor(
                                rh16[b][:, cs], rwk[:], SX, hblk[:],
                                mybir.AluOpType.mult, mybir.AluOpType.mult)
                        else:
                            nc.scalar.activation(z16[b][:, cs], ps[:],
                                                 AF.Sigmoid, bias=bzt[:],
                                                 scale=1.0 / SPS)

            nc.leave_named_scope("mlp_rz", sc_rz[0], False)

            # ---------------- diffusion 2 (rh chain) ----------------
            def build_nm2():
                nms = []
                for jp in range(NJP):
                    ps = pst_pool.tile([128, 2, 512], BF16, name="t", tag="tr")
                    for h in range(2):
                        it = 2 * jp + h
                        for b in range(C):
                            nc.tensor.transpose(
                                ps[:, h, 128 * b:128 * (b + 1)],
                                rh16[b][:, 128 * it:128 * (it + 1)],
                                identb[:])
                    t = nm2_pool.tile([128, 2, 512], FP8, name="t", tag="nm2")
                    nc.vector.tensor_copy(t[:], ps[:])
                    nms.append(t)
                return nms

            with nc.named_scope("d2_hops"):
                for dirw, WP in ((0, WFP), (1, WBP)):
                    ws = load_wdir(WP)
                    cur = build_nm2()
                    for k in range(1, NHOPS + 1):
                        kidx = dirw * NHOPS + k
                        nxt = hop(cur, ws, 4, k,
                                  FEAT2[(kidx - 1) // 2], (kidx - 1) % 2, 0)
                        if k < NHOPS:
                            cur = nxt

            # ---------------- MLP n + final gate ----------------
            sc_n = nc.enter_named_scope("mlp_n", False)
            for b in range(C):
                for blk in range(NBK):
                    cs = slice(512 * blk, 512 * (blk + 1))
                    k0x = k0_pool.tile([64, 512], BF16, name="t", tag="k0b")
                    nc.gpsimd.dma_start(k0x[:], XHK0[b][0:64, cs])
                    fx, fr = [], []
                    for p in range(3):
                        tX = feed_pool.tile([64, 2, 512], FP8, name="t",
                                            tag="fB")
                        nc.scalar.dma_start(
                            tX[:], FEAT1[p][b * FI:b * FI + 64, :, cs])
                        fx.append(tX)
                        tR = feed_pool.tile([128, 2, 512], FP8, name="t",
                                            tag="fA")
                        nc.gpsimd.dma_start(
                            tR[:], FEAT2[p][b * DH:b * DH + 128, :, cs])
                        fr.append(tR)
                    hblk = gw_pool.tile([DH, 512], F32, name="t", tag="h")
                    nc.sync.dma_start(hblk[:], HFM[b][:, cs])
                    ps = ps_pool.tile([128, 512], F32, name="t", tag="hop")
                    nc.tensor.matmul(ps[:], wnk0x[:], k0x[:],
                                     start=True, stop=False)
                    nc.tensor.matmul(ps[:], wnk0r[:], rh16[b][:, cs],
                                     start=False, stop=False)
                    for p in range(3):
                        nc.tensor.matmul(ps[:], wnx[p][:], fx[p][:],
                                         start=False, stop=False, perf_mode=DR)
                        nc.tensor.matmul(ps[:], wnr[p][:], fr[p][:],
                                         start=False, stop=(p == 2),
                                         perf_mode=DR)
                    nf = gw_pool.tile([DH, 512], F32, name="t", tag="nf")
                    nc.scalar.activation(nf[:], ps[:], AF.Tanh, bias=bnt[:],
                                         scale=1.0 / SPS)
                    dlt = gw_pool.tile([DH, 512], F32, name="t", tag="dw")
                    nc.vector.tensor_sub(dlt[:], nf[:], hblk[:])
                    zd = gw_pool.tile([DH, 512], F32, name="t", tag="zd")
                    nc.vector.tensor_mul(zd[:], z16[b][:, cs], dlt[:])
                    o = gw_pool.tile([DH, 512], F32, name="t", tag="o")
                    nc.vector.tensor_add(o[:], zd[:], hblk[:])
                    nc.scalar.dma_start(OUT[b][:, cs], o[:])
            nc.leave_named_scope("mlp_n", sc_n[0], False)

    nc.compile()
    return nc


_NC_CACHE = {}


def _get_nc():
    if "nc" not in _NC_CACHE:
        _NC_CACHE["nc"] = build_nc()
    return _NC_CACHE["nc"]


def _pack_gate_w(W):
    """W [128, 1344] -> (wk0 bf16 [192,128]*8192, wA fp8 [3,128,2,128]*64,
    wB fp8 [3,64,2,128]*64)."""
    W = np.asarray(W, np.float32)
    wk0 = np.ascontiguousarray((W[:, 0:FI].T * SPS)).astype(BF)
    wA = np.zeros((3, 128, 2, DH), np.float32)
    wBt = np.zeros((3, 64, 2, DH), np.float32)
    for p in range(3):
        for h in range(2):
            k = 2 * p + 1 + h
            blkc = W[:, k * FI:(k + 1) * FI]          # [128, 192]
            wA[p, :, h, :] = blkc[:, 0:128].T * SMW
            wBt[p, :, h, :] = blkc[:, 128:192].T * SMW
    return wk0, wA.astype(E4), wBt.astype(E4)


def _pack_n_w(W):
    """Wn [128, 1344] -> k0x bf16 [64,128]*8192, k0r bf16 [128,128]*512,
    wnx fp8 [3,64,2,128]*64, wnr fp8 [3,128,2,128]*64."""
    W = np.asarray(W, np.float32)
    k0x = np.ascontiguousarray(W[:, 0:64].T * SPS).astype(BF)
    k0r = np.ascontiguousarray(W[:, 64:FI].T * (SPS / SX)).astype(BF)
    wnx = np.zeros((3, 64, 2, DH), np.float32)
    wnr = np.zeros((3, 128, 2, DH), np.float32)
    for p in range(3):
        for h in range(2):
            k = 2 * p + 1 + h
            blkc = W[:, k * FI:(k + 1) * FI]
            wnx[p, :, h, :] = blkc[:, 0:64].T * SMW
            wnr[p, :, h, :] = blkc[:, 64:FI].T * SMW
    return k0x, k0r, wnx.astype(E4), wnr.astype(E4)


def _pack_wpair(W):
    """W [N,N] -> fp8 [NJP,128,2,N]: [jp,p,h,i] = 512*W[i, jp*256+h*128+p]."""
    WT = np.asarray(W, np.float32).T * SW                 # [j, i]
    N = WT.shape[0]
    return np.ascontiguousarray(
        WT.reshape(NJP, 2, 128, N).transpose(0, 2, 1, 3)).astype(E4)


def make_in_maps(x, h_prev, W_fwd, W_bwd, Wr, br, Wz, bz, Wn, bn):
    x = np.asarray(x, np.float32)
    h_prev = np.asarray(h_prev, np.float32)
    B, N, Din = x.shape
    wfp = _pack_wpair(W_fwd)
    wbp = _pack_wpair(W_bwd)
    wrk0, wrA, wrB = _pack_gate_w(Wr)
    wzk0, wzA, wzB = _pack_gate_w(Wz)
    wnk0x, wnk0r, wnx, wnr = _pack_n_w(Wn)
    identb = np.eye(128, dtype=np.float32).astype(BF)
    brc = np.ascontiguousarray(np.asarray(br, np.float32).reshape(DH, 1))
    bzc = np.ascontiguousarray(np.asarray(bz, np.float32).reshape(DH, 1))
    bnc = np.ascontiguousarray(np.asarray(bn, np.float32).reshape(DH, 1))
    ncores = B // C
    in_maps = []
    for cix in range(ncores):
        xs = x[C * cix:C * (cix + 1)]
        hs = h_prev[C * cix:C * (cix + 1)]
        xh = np.concatenate([xs, hs], axis=-1)            # [C, N, 192]
        flat = np.ascontiguousarray(xh.transpose(1, 0, 2).reshape(N, C * FI))
        xh_nm8 = np.ascontiguousarray(
            (flat * SX).reshape(NJP, 2, 128, C * FI).transpose(0, 2, 1, 3)
        ).astype(E4)
        xh_k0 = np.ascontiguousarray(xh.transpose(0, 2, 1)).astype(BF)
        h_fm = np.ascontiguousarray(hs.transpose(0, 2, 1))
        in_maps.append(dict(
            xh_nm8=xh_nm8, xh_k0=xh_k0, wfp=wfp, wbp=wbp, h_fm=h_fm,
            wrk0=wrk0, wrA=wrA, wrB=wrB, wzk0=wzk0, wzA=wzA, wzB=wzB,
            wnk0x=wnk0x, wnk0r=wnk0r, wnx=wnx, wnr=wnr,
            br_c=brc, bz_c=bzc, bn_c=bnc, identb=identb))
    return in_maps, ncores


def kernel(x, h_prev, W_fwd, W_bwd, Wr, br, Wz, bz, Wn, bn, _trace=False):
    in_maps, ncores = make_in_maps(
        x, h_prev, W_fwd, W_bwd, Wr, br, Wz, bz, Wn, bn)
    nc = _get_nc()
    res = run_bass_kernel_spmd(nc, in_maps, list(range(ncores)), trace=_trace)
    outs = [np.ascontiguousarray(res.results[c]["out_fm"].transpose(0, 2, 1))
            for c in range(ncores)]
    full = np.concatenate(outs, axis=0).astype(np.float32)
    if _trace:
        return full, res
    return full


# revision 18
# speedup vs baseline: 1.0091x; 1.0091x over previous
"""DCGRU cell on 8 Trainium2 NeuronCores — fp8 DoubleRow edition.

Sharding: data-parallel over batch (B=32 -> 4 per core), adjacency + MLP
weights replicated. No collectives; host gathers per-core outputs.

Key ideas vs the bf16 baseline:
  * Diffusion hop matmuls run in fp8e4 with MatmulPerfMode.DoubleRow: each
    instruction contracts TWO 128-row k-tiles (lhsT [128,2,M], rhs [128,2,N])
    at 0.5 cycles/output-col — 2-4x the bf16 rate.
  * Diffusion 2 only propagates the r*h feature columns (128/batch instead of
    192): the x-part hop features are identical to diffusion 1's and are
    reused for the n-gate MLP. Saves 1/3 of diffusion-2 hop FLOPs.
  * Gate logits accumulate across all 7 k-blocks in a single PSUM group per
    (batch, 512-col block) — no DRAM accumulators, no accum DMAs. Hop
    features spill to DRAM (fp8) and are gathered back per block in paired
    DoubleRow layout.
  * The k=0 MLP segments (the raw x_h / rh features, which dominate logit
    magnitude) stay bf16 with weights pre-scaled by 8192 so they share the
    PSUM accumulation group with the fp8 hop segments.

Scaling scheme (fp8e4 max normal 240):
  x_h, rh stored *16; W stored *512; hop feats stored *128.
  hop1 psum = 16*512*hop  -> copy scale 1/64  -> *128
  hopk psum = 128*512*hop -> copy scale 1/512 -> *128
  MLP hop weights *64 -> logit psum = 128*64 = 8192*logit
  k0 weights: *8192 (vs raw x_h bf16), *512 (vs rh16 bf16)
  activation scale 1/8192 recovers logits.

Per-batch feature order matches the reference concat:
  k-blocks [x_h, Wf^1, Wf^2, Wf^3, Wb^1, Wb^2, Wb^3], 192 feats each.
"""

import sys
import numpy as np
import ml_dtypes

for _p in ("/opt/trn_rl_repo",):
    if _p not in sys.path:
        sys.path.insert(0, _p)

from concourse import bacc, tile, mybir  # noqa: E402
from concourse.bass_utils import run_bass_kernel_spmd  # noqa: E402

F32 = mybir.dt.float32
BF16 = mybir.dt.bfloat16
FP8 = mybir.dt.float8e4
AF = mybir.ActivationFunctionType
DR = mybir.MatmulPerfMode.DoubleRow
E4 = ml_dtypes.float8_e4m3
BF = ml_dtypes.bfloat16

C = 4            # batches per core
FI = 192         # per-batch feature width in d1 (x 64 + h 128)
DH = 128
NCORES = 8
NHOPS = 3
NJP = 8          # node-tile pairs (N = NJP*256)
NBK = 4          # 512-col node blocks
SX = 16.0        # x_h / rh fp8 scale
SW = 512.0       # W fp8 scale
SF = 128.0       # hop-feature fp8 scale
SMW = 64.0       # MLP hop-weight fp8 scale
SPS = SF * SMW   # logit psum scale (8192)


def build_nc():
    N = NJP * 256
    nc = bacc.Bacc("TRN2", target_bir_lowering=False, debug=False,
                   num_devices=NCORES)

    def din(name, shape, dt=F32):
        return nc.dram_tensor(name, shape, dt, kind="ExternalInput").ap()

    XHNM = din("xh_nm8", [NJP, 128, 2, 768], FP8)     # 16*x_h node-major paired
    XHK0 = din("xh_k0", [C, FI, N], BF16)             # x_h feature-major raw
    WFP = din("wfp", [NJP, 128, 2, N], FP8)           # 512*W_fwd^T paired
    WBP = din("wbp", [NJP, 128, 2, N], FP8)
    HFM = din("h_fm", [C, DH, N])                     # h_prev feature-major f32
    # MLP weights (see packer)
    WK0 = {g: din(f"w{g}k0", [FI, DH], BF16) for g in ("r", "z")}
    WA = {g: din(f"w{g}A", [3, 128, 2, DH], FP8) for g in ("r", "z")}
    WB = {g: din(f"w{g}B", [3, 64, 2, DH], FP8) for g in ("r", "z")}
    WNK0X = din("wnk0x", [64, DH], BF16)
    WNK0R = din("wnk0r", [DH, DH], BF16)
    WNX = din("wnx", [3, 64, 2, DH], FP8)
    WNR = din("wnr", [3, 128, 2, DH], FP8)
    BRT = din("br_c", [DH, 1])
    BZT = din("bz_c", [DH, 1])
    BNT = din("bn_c", [DH, 1])
    IDB = din("identb", [128, 128], BF16)
    OUT = nc.dram_tensor("out_fm", [C, DH, N], F32, kind="ExternalOutput").ap()

    # k-pair-adjacent spill layouts: [pair][rows][h][N] so MLP gathers are
    # single 3-D DMAs
    FEAT1 = nc.dram_tensor("feat1", [3, 768, 2, N], FP8).ap()   # d1 feats *128
    FEAT2 = nc.dram_tensor("feat2", [3, C * DH, 2, N], FP8).ap()  # d2 feats

    with tile.TileContext(nc) as tc:
        with (
            tc.tile_pool(name="const", bufs=1) as cpool,
            tc.tile_pool(name="nmx", bufs=8) as nmx_pool,
            tc.tile_pool(name="nm1", bufs=10) as nm1_pool,
            tc.tile_pool(name="nm2", bufs=10) as nm2_pool,
            tc.tile_pool(name="wc", bufs=8) as wc_pool,
            tc.tile_pool(name="fm1", bufs=12) as fm1_pool,
            tc.tile_pool(name="fm2", bufs=8) as fm2_pool,
            tc.tile_pool(name="stg", bufs=9) as stg_pool,
            tc.tile_pool(name="feed", bufs=16) as feed_pool,
            tc.tile_pool(name="k0p", bufs=4) as k0_pool,
            tc.tile_pool(name="gw", bufs=2) as gw_pool,
            tc.tile_pool(name="gres", bufs=4) as gres_pool,
            tc.tile_pool(name="ps", bufs=6, space="PSUM") as ps_pool,
            tc.tile_pool(name="pst", bufs=2, space="PSUM") as pst_pool,
        ):
            # ---------------- constants ----------------
            identb = cpool.tile([128, 128], BF16, tag="idb")
            nc.scalar.dma_start(identb[:], IDB[:])
            # resident paired node-major x_h (chain start for both dirs)
            nm_xh = []
            _qs = [nc.sync, nc.scalar, nc.gpsimd]
            for jp in range(NJP):
                t = nmx_pool.tile([128, 2, 768], FP8, name="t", tag="nmx")
                _qs[jp % 3].dma_start(t[:], XHNM[jp])
                nm_xh.append(t)

            def load_wdir(WP):
                """Block-major striped load: all jp's block-0 slices land
                first so the next hop's first 512-col block starts after
                ~1MB instead of the full 4.2MB."""
                ws = []
                qs = [nc.sync, nc.scalar, nc.gpsimd]
                for jp in range(NJP):
                    t = wc_pool.tile([128, 2, N], FP8, name="t", tag="w")
                    ws.append(t)
                for blk in range(NBK):
                    cs = slice(512 * blk, 512 * (blk + 1))
                    for jp in range(NJP):
                        qs[jp % 3].dma_start(ws[jp][:, :, cs],
                                             WP[jp][:, :, cs])
                return ws

            def hop(cur, ws, nch, k, FEATD, kh, rowbase):
                """One DoubleRow hop with fused per-block retransposition.
                Spills fp8 *SF feats to FEATD[:, kh, :]; for k < NHOPS also
                returns the next chain's paired fp8 nm tiles (transposes for
                jp pair (2b, 2b+1) only need block b's columns)."""
                nms = [] if k < NHOPS else None
                scale = 1.0 / 64.0 if k == 1 else 1.0 / 512.0
                for blk in range(NBK):
                    cs = slice(512 * blk, 512 * (blk + 1))
                    pss = [ps_pool.tile([128, 512], F32, name="t", tag="hop")
                           for _ in range(nch)]
                    for jp in range(NJP):
                        rhs = ws[jp][:, :, cs]
                        for c in range(nch):
                            nc.tensor.matmul(
                                pss[c][:],
                                cur[jp][:, :, 128 * c:128 * (c + 1)],
                                rhs,
                                start=(jp == 0), stop=(jp == NJP - 1),
                                perf_mode=DR)
                    fmb = None
                    if nms is not None:
                        pool = fm1_pool if nch == 6 else fm2_pool
                        tg = "fm1" if nch == 6 else "fm2"
                        fmb = [pool.tile([128, 512], BF16, name="t", tag=tg)
                               for _ in range(nch)]
                    for c in range(nch):
                        stg = stg_pool.tile([128, 512], FP8, name="t",
                                            tag="stg")
                        nc.scalar.activation(stg[:], pss[c][:], AF.Copy,
                                             scale=scale)
                        nc.sync.dma_start(
                            FEATD[rowbase + 128 * c:rowbase + 128 * (c + 1),
                                  kh, cs], stg[:])
                        if fmb is not None:
                            nc.vector.tensor_scalar_mul(fmb[c][:], pss[c][:],
                                                        scale)
                    if nms is not None:
                        for q in range(2):
                            t = (nm1_pool.tile([128, 2, 768], FP8, name="t",
                                               tag="nm1") if nch == 6 else
                                 nm2_pool.tile([128, 2, 512], FP8, name="t",
                                               tag="nm2"))
                            for h in range(2):
                                ps = pst_pool.tile([128, 128 * nch], BF16,
                                                   name="t", tag="tr")
                                for c in range(nch):
                                    nc.tensor.transpose(
                                        ps[:, 128 * c:128 * (c + 1)],
                                        fmb[c][:, 128 * (2 * q + h):
                                               128 * (2 * q + h + 1)],
                                        identb[:])
                                nc.vector.tensor_copy(t[:, h, :], ps[:])
                            nms.append(t)
                return nms

            # ---------------- diffusion 1 ----------------
            with nc.named_scope("d1_hops"):
                for dirw, WP in ((0, WFP), (1, WBP)):
                    ws = load_wdir(WP)
                    cur = nm_xh
                    for k in range(1, NHOPS + 1):
                        kidx = dirw * NHOPS + k  # 1..6
                        nxt = hop(cur, ws, 6, k,
                                  FEAT1[(kidx - 1) // 2], (kidx - 1) % 2, 0)
                        if k < NHOPS:
                            cur = nxt

            # ---- deferred small-const loads (weights/biases for MLPs) ----
            brt = cpool.tile([DH, 1], F32, tag="brt")
            nc.scalar.dma_start(brt[:], BRT[:])
            bzt = cpool.tile([DH, 1], F32, tag="bzt")
            nc.scalar.dma_start(bzt[:], BZT[:])
            bnt = cpool.tile([DH, 1], F32, tag="bnt")
            nc.scalar.dma_start(bnt[:], BNT[:])
            wk0a, wk0b, wa, wb = {}, {}, {}, {}
            for g in ("r", "z"):
                wk0a[g] = cpool.tile([128, DH], BF16, name="t", tag=f"w{g}k0a")
                nc.scalar.dma_start(wk0a[g][:], WK0[g][0:128, :])
                wk0b[g] = cpool.tile([64, DH], BF16, name="t", tag=f"w{g}k0b")
                nc.scalar.dma_start(wk0b[g][:], WK0[g][128:192, :])
                wa[g] = []
                wb[g] = []
                for p in range(3):
                    t = cpool.tile([128, 2, DH], FP8, name="t", tag=f"w{g}A{p}")
                    nc.scalar.dma_start(t[:], WA[g][p])
                    wa[g].append(t)
                    t = cpool.tile([64, 2, DH], FP8, name="t", tag=f"w{g}B{p}")
                    nc.scalar.dma_start(t[:], WB[g][p])
                    wb[g].append(t)
            wnk0x = cpool.tile([64, DH], BF16, tag="wnk0x")
            nc.scalar.dma_start(wnk0x[:], WNK0X[:])
            wnk0r = cpool.tile([DH, DH], BF16, tag="wnk0r")
            nc.scalar.dma_start(wnk0r[:], WNK0R[:])
            wnx, wnr = [], []
            for p in range(3):
                t = cpool.tile([64, 2, DH], FP8, name="t", tag=f"wnx{p}")
                nc.scalar.dma_start(t[:], WNX[p])
                wnx.append(t)
                t = cpool.tile([128, 2, DH], FP8, name="t", tag=f"wnr{p}")
                nc.scalar.dma_start(t[:], WNR[p])
                wnr.append(t)


            # ---------------- MLP r,z + rh ----------------
            sc_rz = nc.enter_named_scope("mlp_rz", False)
            z16 = [gres_pool.tile([DH, N], BF16, name="t", tag="z16")
                   for _ in range(C)]
            rh16 = [gres_pool.tile([DH, N], BF16, name="t", tag="rh16")
                    for _ in range(C)]
            for b in range(C):
                for blk in range(NBK):
                    cs = slice(512 * blk, 512 * (blk + 1))
                    k0a = k0_pool.tile([128, 512], BF16, name="t", tag="k0a")
                    nc.gpsimd.dma_start(k0a[:], XHK0[b][0:128, cs])
                    k0b = k0_pool.tile([64, 512], BF16, name="t", tag="k0b")
                    nc.gpsimd.dma_start(k0b[:], XHK0[b][128:192, cs])
                    fA, fB = [], []
                    for p in range(3):
                        tA = feed_pool.tile([128, 2, 512], FP8, name="t",
                                            tag="fA")
                        nc.gpsimd.dma_start(
                            tA[:], FEAT1[p][b * FI:b * FI + 128, :, cs])
                        fA.append(tA)
                        tB = feed_pool.tile([64, 2, 512], FP8, name="t",
                                            tag="fB")
                        nc.sync.dma_start(
                            tB[:], FEAT1[p][b * FI + 128:b * FI + 192, :, cs])
                        fB.append(tB)
                    hblk = gw_pool.tile([DH, 512], F32, name="t", tag="h")
                    nc.sync.dma_start(hblk[:], HFM[b][:, cs])
                    for g in ("r", "z"):
                        ps = ps_pool.tile([128, 512], F32, name="t", tag="hop")
                        nc.tensor.matmul(ps[:], wk0a[g][:], k0a[:],
                                         start=True, stop=False)
                        nc.tensor.matmul(ps[:], wk0b[g][:], k0b[:],
                                         start=False, stop=False)
                        for p in range(3):
                            nc.tensor.matmul(ps[:], wa[g][p][:], fA[p][:],
                                             start=False, stop=False,
                                             perf_mode=DR)
                            nc.tensor.matmul(ps[:], wb[g][p][:], fB[p][:],
                                             start=False, stop=(p == 2),
                                             perf_mode=DR)
                        if g == "r":
                            rwk = gw_pool.tile([DH, 512], F32, name="t",
                                               tag="rw")
                            nc.scalar.activation(rwk[:], ps[:], AF.Sigmoid,
                                                 bias=brt[:], scale=1.0 / SPS)
                            nc.vector.scalar_tensor_tensor(
                                rh16[b][:, cs], rwk[:], SX, hblk[:],
                                mybir.AluOpType.mult, mybir.AluOpType.mult)
                        else:
                            nc.scalar.activation(z16[b][:, cs], ps[:],
                                                 AF.Sigmoid, bias=bzt[:],
                                                 scale=1.0 / SPS)

            nc.leave_named_scope("mlp_rz", sc_rz[0], False)

            # ---------------- diffusion 2 (rh chain) ----------------
            def build_nm2():
                nms = []
                for jp in range(NJP):
                    ps = pst_pool.tile([128, 2, 512], BF16, name="t", tag="tr")
                    for h in range(2):
                        it = 2 * jp + h
                        for b in range(C):
                            nc.tensor.transpose(
                                ps[:, h, 128 * b:128 * (b + 1)],
                                rh16[b][:, 128 * it:128 * (it + 1)],
                                identb[:])
                    t = nm2_pool.tile([128, 2, 512], FP8, name="t", tag="nm2")
                    nc.vector.tensor_copy(t[:], ps[:])
                    nms.append(t)
                return nms

            with nc.named_scope("d2_hops"):
                for dirw, WP in ((0, WFP), (1, WBP)):
                    ws = load_wdir(WP)
                    cur = build_nm2()
                    for k in range(1, NHOPS + 1):
                        kidx = dirw * NHOPS + k
                        nxt = hop(cur, ws, 4, k,
                                  FEAT2[(kidx - 1) // 2], (kidx - 1) % 2, 0)
                        if k < NHOPS:
                            cur = nxt

            # ---------------- MLP n + final gate ----------------
            sc_n = nc.enter_named_scope("mlp_n", False)
            for b in range(C):
                for blk in range(NBK):
                    cs = slice(512 * blk, 512 * (blk + 1))
                    k0x = k0_pool.tile([64, 512], BF16, name="t", tag="k0b")
                    nc.gpsimd.dma_start(k0x[:], XHK0[b][0:64, cs])
                    fx, fr = [], []
                    for p in range(3):
                        tX = feed_pool.tile([64, 2, 512], FP8, name="t",
                                            tag="fB")
                        nc.sync.dma_start(
                            tX[:], FEAT1[p][b * FI:b * FI + 64, :, cs])
                        fx.append(tX)
                        tR = feed_pool.tile([128, 2, 512], FP8, name="t",
                                            tag="fA")
                        nc.gpsimd.dma_start(
                            tR[:], FEAT2[p][b * DH:b * DH + 128, :, cs])
                        fr.append(tR)
                    hblk = gw_pool.tile([DH, 512], F32, name="t", tag="h")
                    nc.sync.dma_start(hblk[:], HFM[b][:, cs])
                    ps = ps_pool.tile([128, 512], F32, name="t", tag="hop")
                    nc.tensor.matmul(ps[:], wnk0x[:], k0x[:],
                                     start=True, stop=False)
                    nc.tensor.matmul(ps[:], wnk0r[:], rh16[b][:, cs],
                                     start=False, stop=False)
                    for p in range(3):
                        nc.tensor.matmul(ps[:], wnx[p][:], fx[p][:],
                                         start=False, stop=False, perf_mode=DR)
                        nc.tensor.matmul(ps[:], wnr[p][:], fr[p][:],
                                         start=False, stop=(p == 2),
                                         perf_mode=DR)
                    nf = gw_pool.tile([DH, 512], F32, name="t", tag="nf")
                    nc.scalar.activation(nf[:], ps[:], AF.Tanh, bias=bnt[:],
                                         scale=1.0 / SPS)
                    dlt = gw_pool.tile([DH, 512], F32, name="t", tag="dw")
                    nc.vector.tensor_sub(dlt[:], nf[:], hblk[:])
                    zd = gw_pool.tile([DH, 512], F32, name="t", tag="zd")
                    nc.vector.tensor_mul(zd[:], z16[b][:, cs], dlt[:])
                    o = gw_pool.tile([DH, 512], F32, name="t", tag="o")
                    nc.vector.tensor_add(o[:], zd[:], hblk[:])
                    nc.scalar.dma_start(OUT[b][:, cs], o[:])
            nc.leave_named_scope("mlp_n", sc_n[0], False)

    nc.compile()
    return nc


_NC_CACHE = {}


def _get_nc():
    if "nc" not in _NC_CACHE:
        _NC_CACHE["nc"] = build_nc()
    return _NC_CACHE["nc"]


def _pack_gate_w(W):
    """W [128, 1344] -> (wk0 bf16 [192,128]*8192, wA fp8 [3,128,2,128]*64,
    wB fp8 [3,64,2,128]*64)."""
    W = np.asarray(W, np.float32)
    wk0 = np.ascontiguousarray((W[:, 0:FI].T * SPS)).astype(BF)
    wA = np.zeros((3, 128, 2, DH), np.float32)
    wBt = np.zeros((3, 64, 2, DH), np.float32)
    for p in range(3):
        for h in range(2):
            k = 2 * p + 1 + h
            blkc = W[:, k * FI:(k + 1) * FI]          # [128, 192]
            wA[p, :, h, :] = blkc[:, 0:128].T * SMW
            wBt[p, :, h, :] = blkc[:, 128:192].T * SMW
    return wk0, wA.astype(E4), wBt.astype(E4)


def _pack_n_w(W):
    """Wn [128, 1344] -> k0x bf16 [64,128]*8192, k0r bf16 [128,128]*512,
    wnx fp8 [3,64,2,128]*64, wnr fp8 [3,128,2,128]*64."""
    W = np.asarray(W, np.float32)
    k0x = np.ascontiguousarray(W[:, 0:64].T * SPS).astype(BF)
    k0r = np.ascontiguousarray(W[:, 64:FI].T * (SPS / SX)).astype(BF)
    wnx = np.zeros((3, 64, 2, DH), np.float32)
    wnr = np.zeros((3, 128, 2, DH), np.float32)
    for p in range(3):
        for h in range(2):
            k = 2 * p + 1 + h
            blkc = W[:, k * FI:(k + 1) * FI]
            wnx[p, :, h, :] = blkc[:, 0:64].T * SMW
            wnr[p, :, h, :] = blkc[:, 64:FI].T * SMW
    return k0x, k0r, wnx.astype(E4), wnr.astype(E4)


def _pack_wpair(W):
    """W [N,N] -> fp8 [NJP,128,2,N]: [jp,p,h,i] = 512*W[i, jp*256+h*128+p]."""
    WT = np.asarray(W, np.float32).T * SW                 # [j, i]
    N = WT.shape[0]
    return np.ascontiguousarray(
        WT.reshape(NJP, 2, 128, N).transpose(0, 2, 1, 3)).astype(E4)


def make_in_maps(x, h_prev, W_fwd, W_bwd, Wr, br, Wz, bz, Wn, bn):
    x = np.asarray(x, np.float32)
    h_prev = np.asarray(h_prev, np.float32)
    B, N, Din = x.shape
    wfp = _pack_wpair(W_fwd)
    wbp = _pack_wpair(W_bwd)
    wrk0, wrA, wrB = _pack_gate_w(Wr)
    wzk0, wzA, wzB = _pack_gate_w(Wz)
    wnk0x, wnk0r, wnx, wnr = _pack_n_w(Wn)
    identb = np.eye(128, dtype=np.float32).astype(BF)
    brc = np.ascontiguousarray(np.asarray(br, np.float32).reshape(DH, 1))
    bzc = np.ascontiguousarray(np.asarray(bz, np.float32).reshape(DH, 1))
    bnc = np.ascontiguousarray(np.asarray(bn, np.float32).reshape(DH, 1))
    ncores = B // C
    in_maps = []
    for cix in range(ncores):
        xs = x[C * cix:C * (cix + 1)]
        hs = h_prev[C * cix:C * (cix + 1)]
        xh = np.concatenate([xs, hs], axis=-1)            # [C, N, 192]
        flat = np.ascontiguousarray(xh.transpose(1, 0, 2).reshape(N, C * FI))
        xh_nm8 = np.ascontiguousarray(
            (flat * SX).reshape(NJP, 2, 128, C * FI).transpose(0, 2, 1, 3)
        ).astype(E4)
        xh_k0 = np.ascontiguousarray(xh.transpose(0, 2, 1)).astype(BF)
        h_fm = np.ascontiguousarray(hs.transpose(0, 2, 1))
        in_maps.append(dict(
            xh_nm8=xh_nm8, xh_k0=xh_k0, wfp=wfp, wbp=wbp, h_fm=h_fm,
            wrk0=wrk0, wrA=wrA, wrB=wrB, wzk0=wzk0, wzA=wzA, wzB=wzB,
            wnk0x=wnk0x, wnk0r=wnk0r, wnx=wnx, wnr=wnr,
            br_c=brc, bz_c=bzc, bn_c=bnc, identb=identb))
    return in_maps, ncores


def kernel(x, h_prev, W_fwd, W_bwd, Wr, br, Wz, bz, Wn, bn, _trace=False):
    in_maps, ncores = make_in_maps(
        x, h_prev, W_fwd, W_bwd, Wr, br, Wz, bz, Wn, bn)
    nc = _get_nc()
    res = run_bass_kernel_spmd(nc, in_maps, list(range(ncores)), trace=_trace)
    outs = [np.ascontiguousarray(res.results[c]["out_fm"].transpose(0, 2, 1))
            for c in range(ncores)]
    full = np.concatenate(outs, axis=0).astype(np.float32)
    if _trace:
        return full, res
    return full
